# revision 48
# baseline (speedup 1.0000x reference)
"""Trainium2 Bass kernel for Transformer-XL style relative-position MHSA.

Problem: nn_MultiHeadSelfAttention_14989435863450
  B=2, S=2048, D=512, H=8, dh=64, fp32 I/O.

Sharding (8 cores): core c -> batch b = c//4, head pair h0 = 2*(c%4).
Each core computes its 2 heads' attention and the partial output
projection; host sums 4 partials per batch and adds (bv @ Wo + bo).

Math folds (exact):
  - bq folds into u,v:  u_eff = (u + bq) / sqrt(D)
  - bk adds a per-query-row constant to scores -> cancels in softmax
  - bv contributes attn-weighted 1 * bv = bv -> host-side constant
  - 1/sqrt(D) folded into q at evacuation time

v2 design (pipelined waves; see kernel_baseline.py for the v1 layout):
  - Scores built TRANSPOSED (sT[k, q]) so attn@v needs no transpose.
  - Rel-shift via DRAM bounce, but PB is SPLIT at the half boundary:
    PBa[h] rows 0..1151 (q-blocks 0-8) serves half 0's shifted reads,
    PBb[h] rows 1024..2047 (blocks 8-15, block 8 written twice) serves
    half 1.  Reads of one half therefore never chain behind writes for
    the other half (whole-tile dep tracking).
  - Ring assignment: sync HWDGE ring carries ONLY the XBAR transposed
    panel reads (concurrent XBAR transposes on both rings race on the
    shared transpose unit -> single ring).  h0's PB writes ride the
    gpsimd SWDGE queue, h1's the scalar HWDGE ring; input loads + out
    writes also scalar.  This kills the v1 serialization where the
    whole main loop queued behind all 32 PB writes on one ring.
  - 4 waves (h0,f0) (h1,f0) (h0,f1) (h1,f1), each: 16 XBAR panel reads
    issued up front, then 16 kt-steps (content matmul -> +pos panel add
    (DVE) -> exp (Act) -> attn@v two steps late so exp latency is off
    the PE critical path).  Pos blocks 9-15 run early in wave 2 (their
    PBb writes gate wave 3); v-projection fills wave 1; output
    projection for half 0 overlaps waves 3-4, only half 1 remains as
    tail.  Z -> 1/Z runs per (head, half) one wave late (small
    non-XBAR transposed read on the scalar ring).
  - Input x/pos loaded as 4 per-chunk SBUF tiles so projections start
    while later chunks stream; ueff/veff loaded first (the q-evac
    critical chain waits on them).
  - out_partial is bf16 (host accumulates in fp32): halves out traffic.

Known limits (measured): the PE runs mostly at its 1.2GHz mid pstate -
it reaches 2.4GHz only after a ~3us gap-free run and drops back on any
~1.5us+ stall, and full clock appears only when the XBAR/DMA subsystem
is quiet (power-coupled DVFS).  With DVE/Act near-saturated by the
add/exp/evac stream and ~85us of serial XBAR, the span is bounded at
roughly 280-300us; run-to-run device variance is +/-20us.
"""

import math
from contextlib import ExitStack

import numpy as np
import ml_dtypes

import concourse.bass as bass
import concourse.bacc as bacc_mod
import concourse.mybir as mybir
import concourse.tile as tile
from concourse.bass import ts, ds
from concourse.bass_utils import run_bass_kernel_spmd

FP32 = mybir.dt.float32
BF16 = mybir.dt.bfloat16

D_MODEL = 512
NUM_HEADS = 8
D_HEAD = 64
DH2 = 2 * D_HEAD
B_FULL = 2
S_FULL = 2048
P = 128
CH = 512
ISQ = 1.0 / math.sqrt(D_MODEL)

Exp = mybir.ActivationFunctionType.Exp
ADD = mybir.AluOpType.add
MULT = mybir.AluOpType.mult


def build_nc(S=S_FULL):
    nc = bacc_mod.Bacc()
    NB = S // P        # 16 q blocks
    NK = S // P        # 16 k tiles
    NCH = S // CH      # 4 chunks
    KD = D_MODEL // P  # 4
    HALF = S // 2      # 1024
    NBH = NB // 2      # 8 q blocks per half

    xT = nc.declare_dram_parameter("xT", [D_MODEL, S], BF16, isOutput=False)
    posT = nc.declare_dram_parameter("posT", [D_MODEL, S], BF16, isOutput=False)
    Wq = nc.declare_dram_parameter("Wq", [D_MODEL, DH2], BF16, isOutput=False)
    Wk = nc.declare_dram_parameter("Wk", [D_MODEL, DH2], BF16, isOutput=False)
    Wv = nc.declare_dram_parameter("Wv", [D_MODEL, DH2], BF16, isOutput=False)
    Wp = nc.declare_dram_parameter("Wp", [D_MODEL, DH2], BF16, isOutput=False)
    Wo = nc.declare_dram_parameter("Wo", [DH2, D_MODEL], BF16, isOutput=False)
    ueff = nc.declare_dram_parameter("ueff", [DH2, 1], FP32, isOutput=False)
    veff = nc.declare_dram_parameter("veff", [DH2, 1], FP32, isOutput=False)
    out_partial = nc.declare_dram_parameter("out_partial", [S, D_MODEL], BF16, isOutput=True)

    with ExitStack() as ctx:
        tc = ctx.enter_context(tile.TileContext(nc))
        consts = ctx.enter_context(tc.tile_pool(name="consts", bufs=1))
        blk = ctx.enter_context(tc.tile_pool(name="blk", bufs=3))
        spool = ctx.enter_context(tc.tile_pool(name="spool", bufs=30))
        dram = ctx.enter_context(tc.tile_pool(name="dram", bufs=1, space="DRAM"))
        # PSUM (8 banks): psAcc 1x[65,2,512] (2) + psC 3x[128,2,512] (6)
        psAcc = ctx.enter_context(tc.tile_pool(name="psAcc", bufs=1, space="PSUM"))
        psC = ctx.enter_context(tc.tile_pool(name="psC", bufs=3, space="PSUM"))

        # ---- loads: all on the scalar HWDGE ring, chunked for early start
        w_sbs = {}

        def load_w(nm, handle):
            w_sb = consts.tile([P, KD, DH2], BF16, name=f"{nm}_sb")
            nc.scalar.dma_start(w_sb[:], handle.rearrange("(o p) m -> p o m", p=P))
            w_sbs[nm] = w_sb

        xv = xT.rearrange("(o p) s -> p o s", p=P)
        pv = posT.rearrange("(o p) s -> p o s", p=P)
        xc = [consts.tile([P, KD, CH], BF16, name=f"xc{c}") for c in range(NCH)]
        pc = [consts.tile([P, KD, CH], BF16, name=f"pc{c}") for c in range(NCH)]
        # tiny operands FIRST: evac_q (the wave-1 critical chain) waits
        # on ueff/veff, so they must not queue behind the big loads
        ueff_sb = consts.tile([DH2, 1], FP32)
        nc.scalar.dma_start(ueff_sb[:], ueff[:, :])
        veff_sb = consts.tile([DH2, 1], FP32)
        nc.scalar.dma_start(veff_sb[:], veff[:, :])
        load_w("Wq", Wq)
        load_w("Wp", Wp)
        # load order feeds projq g0 (xc0-1), then projp (pc0-3), then
        # projq g1 (xc2-3) with no PE stall in between
        for c in (0, 1):
            nc.scalar.dma_start(xc[c][:], xv[:, :, ts(c, CH)])
        for c in range(NCH):
            nc.scalar.dma_start(pc[c][:], pv[:, :, ts(c, CH)])
        for c in (2, 3):
            nc.scalar.dma_start(xc[c][:], xv[:, :, ts(c, CH)])
        load_w("Wk", Wk)
        load_w("Wv", Wv)
        Wo_sb = consts.tile([D_HEAD, 2, D_MODEL], BF16)
        nc.scalar.dma_start(Wo_sb[:], Wo.rearrange("(h d) n -> d h n", h=2))

        qTu = consts.tile([DH2, S], BF16)
        qTv = consts.tile([DH2, S], BF16)
        kT = consts.tile([DH2, S], BF16)
        pT = consts.tile([DH2, S], BF16)
        vv_aug = consts.tile([P, NK, 2, D_HEAD + 1], BF16)
        ones_st = consts.tile([P, NK * 2], FP32)
        nc.vector.memset(ones_st[:], 1.0)
        nc.vector.tensor_copy(
            vv_aug[:, :, :, D_HEAD : D_HEAD + 1],
            ones_st[:].rearrange("p (a b c) -> p a b c", a=NK, b=2),
        )

        # ---- projections (per group g of 2 chunks) ----
        def proj_group(w_sb, src, g, evac):
            pg = psC.tile([P, 2, CH], FP32, tag="ps", name="pg")
            for j in range(2):
                chn = 2 * g + j
                for kt in range(KD):
                    nc.tensor.matmul(
                        pg[:, j, :],
                        lhsT=w_sb[:, kt, :],
                        rhs=src[chn][:, kt, :],
                        start=(kt == 0),
                        stop=(kt == KD - 1),
                    )
            evac(g, pg)

        def evac_q(g, pg):
            sl = ds(g * 2 * CH, 2 * CH)
            pv_ = pg[:].rearrange("p a b -> p (a b)")
            nc.vector.tensor_scalar(qTu[:, sl], pv_, ISQ, ueff_sb[:, 0:1], MULT, ADD)
            nc.vector.tensor_scalar(qTv[:, sl], pv_, ISQ, veff_sb[:, 0:1], MULT, ADD)

        def evac_to(dst):
            def evac(g, pg):
                sl = ds(g * 2 * CH, 2 * CH)
                nc.scalar.copy(dst[:, sl], pg[:].rearrange("p a b -> p (a b)"))
            return evac

        def proj_v(sg):
            pvv = psC.tile([P, 2, CH], FP32, tag="ps", name="pvv")
            for j in range(2):
                st = 2 * sg + j
                for kt in range(KD):
                    nc.tensor.matmul(
                        pvv[:, j, 0:DH2],
                        lhsT=xc[st // 4][:, kt, ts(st % 4, P)],
                        rhs=w_sbs["Wv"][:, kt, :],
                        start=(kt == 0),
                        stop=(kt == KD - 1),
                    )
            for j in range(2):
                st = 2 * sg + j
                src = pvv[:, j, 0:DH2].rearrange("p (h d) -> p h d", h=2)
                nc.vector.tensor_copy(vv_aug[:, st, :, 0:D_HEAD], src)

        # ---- pos score DRAM buffers, split at the half boundary ----
        # PBa[h]: q rows 0..(NBH+1)*P-1  (blocks 0..8)  -> half-0 reads
        # PBb[h]: q rows HALF..S-1       (blocks 8..15) -> half-1 reads
        PBa = [dram.tile([(NBH + 1) * P, S + 1], BF16, name=f"pba{h}") for h in range(2)]
        PBb = [dram.tile([HALF, S + 1], BF16, name=f"pbb{h}") for h in range(2)]

        def pos_block(ib):
            """pos scores for q rows [128*ib, +128), BOTH heads jointly:
            the two heads' matmuls sit at PE array rows 0-63 and 64-127
            (lhsT partition offset), so consecutive pairs overlap.

            h0's evacs + PB write are emitted FIRST so h0's buffers (the
            gate for the next wave's panel reads) complete before h1's
            trailing work.  Write rings by deadline: PBa[0] (wave-1 gate)
            -> sync, idle until the first panel read which must queue
            behind these writes anyway; PBa[1] (wave-2 gate) + PBb[0]
            (wave 3) -> scalar, behind the loads; PBb[1] (wave 4) ->
            gpsimd SWDGE (slow dispatch, latest deadline)."""
            pes = [
                blk.tile([P, S + 1], BF16, tag=f"posext{h}", bufs=5, name="pe")
                for h in range(2)
            ]
            for h in range(2):
                nc.vector.memset(pes[h][:, 0:1], 0.0)
            pps_all = []
            for g in range(NCH // 2):
                # prologue blocks (<= NBH) borrow the idle psAcc bank
                # pair as a 4th pos-score PSUM slot so the PE never
                # stalls on the depth-3 psC rotation; in-wave blocks
                # (9-15) must not touch psAcc (it holds the live attn@v
                # accumulator)
                pps = [
                    psAcc.tile([P, 2, CH], FP32, tag="po", name="ppa")
                    if (g == 1 and ib <= NBH)
                    else psC.tile([P, 2, CH], FP32, tag="ps", name="pp"),
                    psC.tile([P, 2, CH], FP32, tag="ps", name="pp"),
                ]
                for h in range(2):
                    for j in range(2):
                        chn = 2 * g + j
                        nc.tensor.matmul(
                            pps[h][:, j, :],
                            lhsT=qTv[ds(h * D_HEAD, D_HEAD), ts(ib, P)],
                            rhs=pT[ds(h * D_HEAD, D_HEAD), ts(chn, CH)],
                            start=True,
                            stop=True,
                        )
                pps_all.append(pps)
            for h in range(2):
                for g in range(NCH // 2):
                    dst = pes[h][:, ds(1 + g * 2 * CH, 2 * CH)]
                    src = pps_all[g][h][:].rearrange("p a b -> p (a b)")
                    if (h + g) % 2 == 0:
                        nc.vector.tensor_copy(dst, src)
                    else:
                        nc.scalar.copy(dst, src)
                if ib <= NBH:
                    (nc.sync if h == 0 else nc.scalar).dma_start(
                        PBa[h][ts(ib, P), :], pes[h][:]
                    )
                if ib >= NBH:
                    (nc.scalar if h == 0 else nc.gpsimd).dma_start(
                        PBb[h][ts(ib - NBH, P), :], pes[h][:]
                    )

        # unnormalized attn@v results per head (d rows), Z staged separately
        o2u = {}
        zq = {}
        rz = {}
        for h in range(2):
            o2u[h] = blk.tile([D_HEAD, NCH, CH], BF16, tag=f"o2_{h}", bufs=1, name="o2u")
            zq[h] = blk.tile([1, NCH, CH], BF16, tag=f"zq_{h}", bufs=1, name="zq")
            rz[h] = blk.tile([P, NB], FP32, tag=f"rz_{h}", bufs=1, name="rz")
        zd = dram.tile([2, S], BF16, name="zd")

        def issue_read(h, half, kt):
            """prefetch the shifted+transposed pos panel for (h, half, kt)."""
            sp = spool.tile([P, 2, CH], BF16, tag="spos", name="sp")
            if half == 0:
                flat = PBa[h].flatten()
                qview = flat[ds(S, HALF * S)].rearrange("(q k) -> q k", k=S)
            else:
                flat = PBb[h].flatten()
                qview = flat[ds(HALF, HALF * S)].rearrange("(q k) -> q k", k=S)
            nc.sync.dma_start(sp[:].rearrange("p a b -> p (a b)"),
                              qview[:, ts(kt, P)], transpose=True)
            return sp

        def po_step(h, kt, et, po):
            for j in range(2):
                nc.tensor.matmul(
                    po[:, j, :],
                    lhsT=vv_aug[:, kt, h, :],
                    rhs=et[:, j, :],
                    start=(kt == 0),
                    stop=(kt == NK - 1),
                )

        def kt_step(h, half, kt, po, sp, pending, inter):
            """content scores + exp for k-tile kt; attn@v for k-tile kt-2
            (delayed two steps so exp latency is fully off the PE
            critical path)."""
            ps = psC.tile([P, 2, CH], FP32, tag="ps", name="ps")
            for j in range(2):
                c = 2 * half + j
                nc.tensor.matmul(
                    ps[:, j, :],
                    lhsT=kT[ds(h * D_HEAD, D_HEAD), ts(kt, P)],
                    rhs=qTu[ds(h * D_HEAD, D_HEAD), ts(c, CH)],
                    start=True,
                    stop=True,
                )
            for fn in inter:
                fn()
            if len(pending) >= 2:
                po_step(h, *pending.pop(0), po=po)
            sc = blk.tile([P, 2, CH], BF16, tag="sc", bufs=4, name="sc")
            nc.vector.tensor_tensor(sc[:], ps[:], sp[:], ADD)
            et = blk.tile([P, 2, CH], BF16, tag="et", bufs=4, name="et")
            nc.scalar.activation(et[:], sc[:], Exp)
            pending.append((kt, et))

        def wave(h, half, inter_map):
            po = psAcc.tile([D_HEAD + 1, 2, CH], FP32, tag="po", name="po")
            sps = {kt: issue_read(h, half, kt) for kt in range(NK)}
            pending = []
            for kt in range(NK):
                kt_step(h, half, kt, po, sps.pop(kt), pending,
                        inter_map.get(kt, ()))
            for kt, et in pending:
                po_step(h, kt, et, po)
            # evac: d rows -> o2u (DVE), Z row -> zq (Act)
            nc.vector.tensor_copy(o2u[h][:, ts(half, 2), :], po[0:D_HEAD])
            nc.scalar.copy(zq[h][:, ts(half, 2), :], po[D_HEAD : D_HEAD + 1])

        def finish_half(h, half):
            """Z (one half) -> DRAM -> transposed [128, 8] -> rz = 1/Z.
            The [8, 128] source takes dma_start_transpose's small-input
            fallback (plain strided DMA, no XBAR), so the scalar ring is
            safe; emitted in the NEXT wave so its latency is hidden."""
            nc.scalar.dma_start(
                zd[h : h + 1, ds(half * HALF, HALF)],
                zq[h][:, ts(half, 2), :].rearrange("p a b -> p (a b)"),
            )
            zview = zd.flatten()[ds(h * S + half * HALF, HALF)].rearrange(
                "(a b) -> a b", b=P
            )
            rzt = blk.tile([P, NBH], BF16, tag="rzt", name="rzt")
            nc.scalar.dma_start(rzt[:], zview.rearrange("a b -> b a"))
            nc.vector.reciprocal(rz[h][:, ts(half, NBH)], rzt[:])

        def pw_block(ib):
            c, j = ib // NCH, ib % NCH
            pw = psC.tile([P, 2, CH], FP32, tag="ps", name="pw")
            for h in range(2):
                nc.tensor.matmul(
                    pw[:, h, :],
                    lhsT=o2u[h][:, c, ts(j, P)],
                    rhs=Wo_sb[:, h, :],
                    start=True,
                    stop=True,
                )
            t1 = blk.tile([P, D_MODEL], FP32, tag="t1", name="t1")
            nc.scalar.mul(t1[:], pw[:, 1, :], rz[1][:, ib : ib + 1])
            fin = blk.tile([P, D_MODEL], BF16, tag="fin", name="fin")
            nc.vector.scalar_tensor_tensor(
                fin[:], pw[:, 0, :], rz[0][:, ib : ib + 1], t1[:], MULT, ADD
            )
            nc.scalar.dma_start(out_partial[ts(ib, P), :], fin[:])

        # ---- prologue: pos blocks 0-8 (wave 1-2 gates).  Projections are
        # slotted BETWEEN pos blocks as PE gap-fillers: the pos pipeline
        # is evac-bound (~2.3us/block) while its PE work is ~1.2us, and
        # any PE gap >~1.5us drops the PE clock 2.4GHz -> 1.2GHz for the
        # rest of the run.  Order keeps every PE gap under ~1us. ----
        proj_group(w_sbs["Wq"], xc, 0, evac_q)
        proj_group(w_sbs["Wp"], pc, 0, evac_to(pT))
        proj_group(w_sbs["Wp"], pc, 1, evac_to(pT))
        pos_block(0)
        proj_group(w_sbs["Wk"], xc, 0, evac_to(kT))  # kt 0-7 content
        for ib in range(1, 5):
            pos_block(ib)
        proj_group(w_sbs["Wq"], xc, 1, evac_q)   # qTv g1 needed by block 8+
        for ib in range(5, NBH + 1):
            pos_block(ib)

        # ---- waves ----
        w1_inter = {
            0: (lambda: proj_v(0), lambda: proj_v(1)),
            2: (lambda: proj_v(2),),
            4: (lambda: proj_v(3),
                lambda: proj_group(w_sbs["Wk"], xc, 1, evac_to(kT))),
            6: (lambda: proj_v(4),),
            8: (lambda: proj_v(5),),
            11: (lambda: proj_v(6),),
            14: (lambda: proj_v(7),),
        }
        # pos blocks 9-15 early in wave 2 so PBb[0] (wave 3's gate) lands
        # before wave 2 drains; finish_half one wave late hides the Z
        # bounce; pw blocks for half 0 run in waves 3-4 once both heads'
        # half-0 o2u and rz are ready
        w2_inter = {
            0: (lambda: pos_block(9),),
            1: (lambda: pos_block(10),),
            2: (lambda: pos_block(11),),
            3: (lambda: finish_half(0, 0), lambda: pos_block(12)),
            4: (lambda: pos_block(13),),
            5: (lambda: pos_block(14),),
            6: (lambda: pos_block(15),),
        }
        w3_inter = {
            3: (lambda: finish_half(1, 0),),
            8: (lambda: pw_block(0),),
            10: (lambda: pw_block(1),),
            12: (lambda: pw_block(2),),
            14: (lambda: pw_block(3),),
        }
        w4_inter = {
            3: (lambda: finish_half(0, 1),),
            6: (lambda: pw_block(4),),
            9: (lambda: pw_block(5),),
            12: (lambda: pw_block(6),),
            15: (lambda: pw_block(7),),
        }
        wave(0, 0, w1_inter)
        wave(1, 0, w2_inter)
        wave(0, 1, w3_inter)
        wave(1, 1, w4_inter)

        finish_half(1, 1)
        for ib in range(NBH, NB):
            pw_block(ib)

    nc.finalize()
    return nc


# ---------------- host side ----------------

_NC_CACHE = {}


def _get_nc(S=S_FULL):
    if S not in _NC_CACHE:
        _NC_CACHE[S] = build_nc(S)
    return _NC_CACHE[S]


def make_in_maps(inputs, S=S_FULL, n_cores=8):
    bf16 = ml_dtypes.bfloat16
    x = np.asarray(inputs["x"], np.float32)
    pos = np.asarray(inputs["pos_embedding"], np.float32)
    Wq = np.asarray(inputs["Wq"], np.float32)
    bq = np.asarray(inputs["bq"], np.float32)
    Wk = np.asarray(inputs["Wk"], np.float32)
    Wv = np.asarray(inputs["Wv"], np.float32)
    Wp = np.asarray(inputs["Wp"], np.float32)
    u = np.asarray(inputs["u"], np.float32)
    v = np.asarray(inputs["v"], np.float32)
    Wo = np.asarray(inputs["Wo"], np.float32)

    xTb = [np.ascontiguousarray(x[b, :S].T).astype(bf16) for b in range(B_FULL)]
    posTb = [np.ascontiguousarray(pos[b, :S].T).astype(bf16) for b in range(B_FULL)]

    in_maps = []
    for c in range(n_cores):
        b = c // 4
        h0 = 2 * (c % 4)
        sl = slice(h0 * D_HEAD, (h0 + 2) * D_HEAD)
        u_eff = ((u[h0 : h0 + 2].reshape(-1) + bq[sl]) * ISQ).astype(np.float32)
        v_eff = ((v[h0 : h0 + 2].reshape(-1) + bq[sl]) * ISQ).astype(np.float32)
        in_maps.append(
            {
                "xT": xTb[b],
                "posT": posTb[b],
                "Wq": np.ascontiguousarray(Wq[:, sl]).astype(bf16),
                "Wk": np.ascontiguousarray(Wk[:, sl]).astype(bf16),
                "Wv": np.ascontiguousarray(Wv[:, sl]).astype(bf16),
                "Wp": np.ascontiguousarray(Wp[:, sl]).astype(bf16),
                "Wo": np.ascontiguousarray(Wo[sl, :]).astype(bf16),
                "ueff": u_eff.reshape(DH2, 1),
                "veff": v_eff.reshape(DH2, 1),
            }
        )
    return in_maps


def assemble(inputs, results, S=S_FULL):
    bv = np.asarray(inputs["bv"], np.float64)
    Wo = np.asarray(inputs["Wo"], np.float64)
    bo = np.asarray(inputs["bo"], np.float64)
    const = (bv @ Wo + bo).astype(np.float32)
    out = np.zeros((B_FULL, S, D_MODEL), np.float32)
    for c, res in enumerate(results):
        out[c // 4] += np.asarray(res["out_partial"], dtype=np.float32)
    out += const[None, None, :]
    return out


def _run(inputs, trace=False, **kw):
    nc = _get_nc(S_FULL)
    in_maps = make_in_maps(inputs, S_FULL)
    res = run_bass_kernel_spmd(nc, in_maps, list(range(8)), trace=trace, **kw)
    out = assemble(inputs, res.results, S_FULL)
    return out, res


def kernel(**inputs) -> np.ndarray:
    out, _ = _run(inputs, trace=False)
    return out
